# revision 16
# baseline (speedup 1.0000x reference)
"""MoE layer kernel for 8x TRN2 NeuronCores (Bass/Tile).

Math (reference):
    w      = softmax(x @ gate_W + gate_b, axis=-1)[:E]          # [E, F]
    W_eff  = einsum('ef,edf->df', w, expert_W)                  # [D, F]
    b_eff  = einsum('ef,ef->f',  w, expert_b)                   # [F]
    out    = x @ W_eff + b_eff                                  # [N, F]

Sharding: F-parallel across 8 cores (each core owns 128 f-columns).

Key measured facts this implementation is built around:
  - fp16-stationary x fp8e4m3-moving PE matmul is exact; 512-col moving
    costs 213 ns at the full 2.4 GHz pstate when streamed gaplessly.
  - Each dma_start costs ~0.6-0.7 us of issuing-sequencer time, and a
    consumer can only start when its WHOLE transfer lands -> the
    expert_W stream uses per-e-tile (256 KB) DMAs on the Sync ring:
    fine enough to pipeline within a d-super-group, coarse enough that
    the issue rate (8 x 0.61 us per dsg) outruns PE (6.8 us per dsg).
  - The Scalar ring carries NO DMAs (the softmax exp chain paces the
    gate; DMA triggers on its sequencer would delay it by ~0.6 us
    each).  xT-rest rides the Vector ring; expert_b/corr/wq ride the
    otherwise idle GpSimd ring.
  - scalar_tensor_tensor has NO DVE perf modes -> fp8 in0 costs the
    same as fp16; one [128f, 1024e] STT per d (~1.13 us) computes a
    W_eff^T column outright.  NDVE=16 tail d's go this way, leaving
    DVE (~93 us) comfortably under PE so the diagonal-extract chase
    never stalls PE's PSUM-bank recycling.
  - fp8 systematic quantization error is removed by a host-precomputed
    correction: corrT[f, d] = sum_e(W - W8)[d, f] / E (w_e ~ 1/E makes
    sum_e w_e (W - W8)_e ~ mean); residual fluctuation ~0.8% rel.

Device algorithm per core:
  1. Gate GEMM (tokens 0..1023 == experts) on PE from host-transposed
     xT + gw; exp on ACT in two 512-col psum halves (rsA+rsB summed on
     DVE); normalize -> wnorm16 [e_p, et, f] fp16; wnT16 [f_p, e] via
     PE transposes through a 1-bank psum ring.  dsg0 of the diag phase
     is emitted et-major, interleaved into the gate, so PE fills the
     softmax-chain latency with diag work.
  2. W_eff^T, d < DPE: per d, psum[f, f'] = sum_e wnorm[e, f] *
     ew8[e, d, f'] (8 e-tile matmuls; moving tile covers 4 d -> psum
     [128, 512]); DVE extracts each d's diagonal via STT against
     identity, dequant 1/64 folded into the scalar operand.
  3. W_eff^T, d >= DPE: one DVE STT per d (fp8 [f, e] tiles x wnT16),
     accum straight into wefft[:, d]; ~1 interleaved per dsg.
  4. b_eff: one DVE STT (ebT fp16 [f, e] x wnT16) -> beff_col.
  5. wefft += corrT; transpose -> W_eff [d_p, f] fp16; out^T =
     W_eff^T @ xT on PE; bias fused into the psum->SBUF copies
     (alternating ACT/DVE); chunked DMA out (fp16).

All phases share ONE PSUM pool (lp 2x1 + wp 5x1 + tp 1 = 8 banks) —
pool open/close barriers between phases cost ~3-5 us each otherwise.

NOTE: this walrus build rejects any instruction carrying more than ONE
semaphore wait ("Too many sync wait commands"). _split_multi_waits()
post-processes the scheduled program, hoisting extra waits onto
standalone EventSemaphore instructions on the same engine queue.
"""

import numpy as np

N, D, E, F = 4096, 256, 1024, 1024
NCORES = 8
FSH = F // NCORES  # 128 f-columns per core
P = 128
SCALE_W = 64.0     # global fp8 scale for expert_W
DPE = 240          # d < DPE on the PE diag path (15 dsgs of 16)
NDVE = D - DPE     # 16 d's on the DVE STT path

_CACHE = {}
LAST_RESULT = None


def _split_multi_waits(nc):
    """Split multi-wait instructions into chains of single-wait ones."""
    import concourse.mybir as mybir

    n = 0
    for fn in nc.m.functions:
        for bb in fn.blocks:
            out = []
            changed = False
            for ins in bb.instructions:
                si = ins.sync_info
                if si is not None and si.on_wait and len(si.on_wait) > 1:
                    waits = list(si.on_wait)
                    for w in waits[:-1]:
                        es = mybir.InstEventSemaphore(
                            name=f"wsplit_{n}",
                            engine=ins.engine,
                            sync_info=mybir.SyncInfo(
                                on_wait=[w], on_update=[]),
                        )
                        out.append(es)
                        n += 1
                    ins.sync_info = mybir.SyncInfo(
                        on_wait=[waits[-1]], on_update=list(si.on_update))
                    changed = True
                out.append(ins)
            if changed:
                bb.instructions = out
    return n


def _build_bass(has_gb=False, dsg_size=16, prefetch_dsgs=5):
    import concourse.bass as bass
    import concourse.mybir as mybir
    from concourse.masks import make_identity
    from concourse.tile import TileContext

    f32 = mybir.dt.float32
    f16 = mybir.dt.float16
    f8 = mybir.dt.float8e4
    AF = mybir.ActivationFunctionType
    mult = mybir.AluOpType.mult
    add = mybir.AluOpType.add

    nc = bass.Bass(trn_type="TRN2", name="moe_fshard",
                   dynamic_dma_scratch_size=4096)

    xt_d = nc.dram_tensor("xt", [P, 2 * N], f16, kind="ExternalInput")
    gw_d = nc.dram_tensor("gw", [D, F], f16, kind="ExternalInput")
    if has_gb:
        gb_d = nc.dram_tensor("gb", [1, F], f16, kind="ExternalInput")
    ew_d = nc.dram_tensor("ew8", [E, DPE, FSH], f8, kind="ExternalInput")
    wq_d = nc.dram_tensor("wq8", [NDVE // 4, P, 4 * E], f8,
                          kind="ExternalInput")
    corr_d = nc.dram_tensor("corrt", [P, D], f32, kind="ExternalInput")
    eb_d = nc.dram_tensor("ebt", [P, E], f16, kind="ExternalInput")
    out_d = nc.dram_tensor("outT", [FSH, N], f16, kind="ExternalOutput")

    EBLK = E // P            # 8 e-tiles (gate token blocks)
    DH = D // P              # 2 halves of d
    NDSG = DPE // dsg_size   # 15 d super-groups on PE
    NBANK = dsg_size * P // 512   # 4 psum banks per super-group (4 d each)
    GN = EBLK * P            # 1024 gate tokens
    NWQ = NDVE // 4          # 4 wq tile groups

    with TileContext(nc) as tc:
        with tc.tile_pool(name="persist", bufs=1) as persist, \
             tc.tile_pool(name="wep", bufs=1 + prefetch_dsgs) as wep, \
             tc.tile_pool(name="wqp", bufs=NWQ) as wqp, \
             tc.tile_pool(name="ps8", bufs=1, space="PSUM") as ps8:

            # smalls: [:,0:128] identity f32; [:,128:129] beff_col
            smalls = persist.tile([P, 160], f32)
            ident = smalls[:, 0:128]
            beff_col = smalls[:, 128:129]
            sm16 = persist.tile([P, P], f16)
            ident16 = sm16[:, 0:P]

            xT = persist.tile([P, DH, N], f16)          # 16KB/part
            gw_sb = persist.tile([P, DH, F], f16)       # 4KB/part
            wnorm16 = persist.tile([P, EBLK, FSH], f16)  # 2KB/part
            wnT16 = persist.tile([P, E], f16)           # 2KB/part
            wefft = persist.tile([P, D], f32)           # 1KB/part
            weffc = persist.tile([P, D], f32)           # 1KB/part
            corrT = persist.tile([P, D], f32)           # 1KB/part
            weff = persist.tile([P, DH, FSH], f16)      # 0.5KB/part
            ebT_sb = persist.tile([P, E], f16)          # 2KB/part
            scr = persist.tile([P, 4], f32)
            rsA = scr[:, 0:1]
            rsB = scr[:, 1:2]
            rsum = scr[:, 2:3]
            rcp = scr[:, 3:4]
            expsc = persist.tile([P, F], f32)           # 4KB/part
            junk = persist.tile([P, P], f32)
            junk16 = persist.tile([P, E], f16)
            outT_sb = persist.tile([P, N], f16)         # 8KB/part
            if has_gb:
                gb_sb = persist.tile([1, F], f16)
                ones16 = persist.tile([1, P], f16)

            # ---- Sync ring: gate-critical inputs, then the ew8 stream
            nc.sync.dma_start(out=xT[:, 0, 0:GN], in_=xt_d[:, 0:GN])
            nc.sync.dma_start(
                out=gw_sb[:], in_=gw_d.rearrange("(h p) f -> p h f", p=P))
            nc.sync.dma_start(out=xT[:, 1, 0:GN],
                              in_=xt_d[:, N:N + GN])
            if has_gb:
                nc.sync.dma_start(out=gb_sb[:], in_=gb_d[:, :])

            wet_tiles = {}
            ew_v = ew_d.rearrange("(t p) d f -> p t d f", p=P)

            def issue_dsg(dsg, nsplit=1):
                w = wep.tile([P, EBLK, dsg_size, FSH], f8, tag="we",
                             name=f"wet_{dsg}")
                src = ew_v[:, :, dsg * dsg_size:(dsg + 1) * dsg_size, :]
                step = EBLK // nsplit
                for s in range(nsplit):
                    nc.sync.dma_start(out=w[:, s * step:(s + 1) * step],
                                      in_=src[:, s * step:(s + 1) * step])
                wet_tiles[dsg] = w

            issue_dsg(0, nsplit=4)
            for dsg in range(1, prefetch_dsgs):
                issue_dsg(dsg)

            # ---- GpSimd ring: corrT + the wq stream (expert_b after
            # the wq tiles: its consumer runs at the end of the diag)
            nc.gpsimd.dma_start(out=corrT[:], in_=corr_d[:, :])
            wq_tiles = {}

            def issue_wq(g):
                w = wqp.tile([P, 4, E], f8, tag="wq", name=f"wq_{g}")
                nc.gpsimd.dma_start(out=w[:], in_=wq_d[g, :, :])
                wq_tiles[g] = w

            issue_wq(0)
            issue_wq(1)
            nc.gpsimd.dma_start(out=ebT_sb[:], in_=eb_d[:, :])

            # constants (gpsimd iota), after the DMA issues
            make_identity(nc, ident)
            nc.scalar.copy(ident16[:], ident)
            if has_gb:
                nc.vector.memset(ones16[:], 1.0)

            def dve_stt(j):
                """DVE einsum for d = DPE + j."""
                nc.vector.scalar_tensor_tensor(
                    out=junk16[:],
                    in0=wq_tiles[j // 4][:, j % 4, :],
                    scalar=1.0 / SCALE_W, in1=wnT16[:],
                    op0=mult, op1=mult,
                    accum_out=wefft[:, DPE + j:DPE + j + 1])

            def extract(bank, di, d):
                nc.vector.scalar_tensor_tensor(
                    out=junk[:],
                    in0=bank[:, di * P:(di + 1) * P],
                    scalar=1.0 / SCALE_W, in1=ident,
                    op0=mult, op1=mult,
                    accum_out=wefft[:, d:d + 1])

            # ====== Phase 1+2 interleaved: gate + diag dsg0 =========
            ETLAG = 4
            banks0 = [ps8.tile([P, 512], f32, tag="wp", bufs=5,
                               name=f"wp_0_{b}") for b in range(NBANK)]

            def diag_et(banks, wet, et):
                for b in range(NBANK):
                    nc.tensor.matmul(
                        banks[b][:],
                        wnorm16[:, et, :],
                        wet[:, et, b * 4:(b + 1) * 4, :],
                        start=(et == 0),
                        stop=(et == EBLK - 1))

            def wn_transpose(a):
                tp = ps8.tile([P, P], f16, tag="tp", bufs=1,
                              name=f"tp_{a}")
                nc.tensor.transpose(tp[:], wnorm16[:, a, :], ident16)
                nc.scalar.copy(wnT16[:, a * P:(a + 1) * P], tp[:])

            for a in range(EBLK):
                asl = slice(a * P, (a + 1) * P)
                lpA = ps8.tile([P, 512], f32, tag="lp", bufs=2,
                               name=f"lpA_{a}")
                lpB = ps8.tile([P, 512], f32, tag="lp", bufs=2,
                               name=f"lpB_{a}")
                for lp, cc in ((lpA, 0), (lpB, 512)):
                    nc.tensor.matmul(lp[:],
                                     xT[:, 0, asl],
                                     gw_sb[:, 0, cc:cc + 512],
                                     start=True, stop=False)
                    nc.tensor.matmul(lp[:],
                                     xT[:, 1, asl],
                                     gw_sb[:, 1, cc:cc + 512],
                                     start=False, stop=not has_gb)
                    if has_gb:
                        nc.tensor.matmul(lp[:], ones16[:],
                                         gb_sb[0:1, cc:cc + 512],
                                         start=False, stop=True)
                nc.scalar.activation(expsc[:, 0:512], lpA[:], AF.Exp,
                                     accum_out=rsA)
                nc.scalar.activation(expsc[:, 512:1024], lpB[:], AF.Exp,
                                     accum_out=rsB)
                nc.vector.scalar_tensor_tensor(
                    out=rsum, in0=rsA, scalar=1.0, in1=rsB,
                    op0=mult, op1=add)
                nc.vector.reciprocal(rcp, rsum)
                nc.vector.tensor_scalar_mul(
                    wnorm16[:, a, :], expsc[:, 0:FSH], rcp)
                wn_transpose(a)
                if a >= ETLAG:
                    diag_et(banks0, wet_tiles[0], a - ETLAG)
            for et in range(EBLK - ETLAG, EBLK):
                diag_et(banks0, wet_tiles[0], et)
            wet_tiles.pop(0)

            ndve_done = 0

            def bank_finish(banks, dsg):
                nonlocal ndve_done
                for b in range(NBANK):
                    for di in range(4):
                        extract(banks[b], di,
                                dsg * dsg_size + b * 4 + di)
                if dsg < 3:
                    tgt = 0
                else:
                    tgt = min(NDVE,
                              ((dsg - 2) * NDVE) // (NDSG - 3))
                while ndve_done < tgt:
                    dve_stt(ndve_done)
                    g = ndve_done // 4
                    if g + 2 < NWQ and ndve_done % 4 == 0:
                        issue_wq(g + 2)
                    ndve_done += 1

            bank_finish(banks0, 0)

            # ====== Phase 2 cont.: dsg 1..14, bank-major ============
            for dsg in range(1, NDSG):
                if dsg + prefetch_dsgs <= NDSG:
                    issue_dsg(dsg + prefetch_dsgs - 1)
                if dsg == 6:
                    nc.sync.dma_start(out=xT[:, 0, GN:N],
                                      in_=xt_d[:, GN:N])
                if dsg == 10:
                    nc.sync.dma_start(out=xT[:, 1, GN:N],
                                      in_=xt_d[:, N + GN:2 * N])
                banks = [ps8.tile([P, 512], f32, tag="wp", bufs=5,
                                  name=f"wp_{dsg}_{b}")
                         for b in range(NBANK)]
                wet = wet_tiles.pop(dsg)
                for b in range(NBANK):
                    for et in range(EBLK):
                        nc.tensor.matmul(
                            banks[b][:],
                            wnorm16[:, et, :],
                            wet[:, et, b * 4:(b + 1) * 4, :],
                            start=(et == 0),
                            stop=(et == EBLK - 1))
                bank_finish(banks, dsg)
                if dsg == 8:
                    # d < 128 fully extracted: do half the correction +
                    # transpose now, shortening the tail
                    nc.vector.scalar_tensor_tensor(
                        out=weffc[:, 0:P], in0=wefft[:, 0:P],
                        scalar=1.0, in1=corrT[:, 0:P],
                        op0=mult, op1=add)
                    pt3e = ps8.tile([P, 512], f32, tag="wp", bufs=5,
                                    name="pt3_early")
                    nc.tensor.transpose(
                        pt3e[:, 0:P], weffc[:, 0:P], ident)
                    nc.scalar.copy(weff[:, 0, :], pt3e[:, 0:P])
            while ndve_done < NDVE:
                dve_stt(ndve_done)
                ndve_done += 1

            # b_eff: one DVE STT against wnT16 (only needed by the
            # GEMM-copy bias, so it runs after the extract chase)
            nc.vector.scalar_tensor_tensor(
                out=junk16[:], in0=ebT_sb[:], scalar=1.0, in1=wnT16[:],
                op0=mult, op1=mult, accum_out=beff_col)

            # fp8 mean-error correction (half 1; half 0 done mid-diag)
            nc.vector.scalar_tensor_tensor(
                out=weffc[:, P:D], in0=wefft[:, P:D], scalar=1.0,
                in1=corrT[:, P:D], op0=mult, op1=add)

            # ====== Phase 3: W_eff transpose + final GEMM ==========
            pt3 = ps8.tile([P, 512], f32, tag="wp", bufs=5,
                           name="pt3_1")
            nc.tensor.transpose(pt3[:, 0:P], weffc[:, P:D], ident)
            nc.scalar.copy(weff[:, 1, :], pt3[:, 0:P])
            for ch in range(N // 512):
                sl = slice(ch * 512, (ch + 1) * 512)
                ps = ps8.tile([P, 512], f32, tag="wp", bufs=5,
                              name=f"fp_{ch}")
                nc.tensor.matmul(ps[:], weff[:, 0, :],
                                 xT[:, 0, sl],
                                 start=True, stop=False)
                nc.tensor.matmul(ps[:], weff[:, 1, :],
                                 xT[:, 1, sl],
                                 start=False, stop=True)
                if ch % 2 == 0:
                    nc.scalar.activation(outT_sb[:, sl], ps[:],
                                         AF.Identity, bias=beff_col,
                                         scale=1.0)
                else:
                    nc.vector.tensor_scalar_add(
                        out=outT_sb[:, sl], in0=ps[:],
                        scalar1=beff_col)
                nc.sync.dma_start(out=out_d[:, sl],
                                  in_=outT_sb[:, sl])

    _split_multi_waits(nc)
    return nc


def _prep_in_maps(x, gate_W, gate_b, expert_W, expert_b):
    import ml_dtypes
    f8 = ml_dtypes.float8_e4m3

    x16 = np.asarray(x).astype(np.float16)
    # host transpose: xT[p, h, n] = x[n, h*128 + p]
    xt = np.ascontiguousarray(
        x16.T.reshape(2, P, N).transpose(1, 0, 2).reshape(P, 2 * N))
    gate_W = np.asarray(gate_W, dtype=np.float32)
    gate_b = np.asarray(gate_b, dtype=np.float32).reshape(1, F)
    has_gb = bool(np.any(gate_b))
    expert_W = np.asarray(expert_W, dtype=np.float32)
    expert_b = np.asarray(expert_b, dtype=np.float32)

    in_maps = []
    for c in range(NCORES):
        sh = slice(c * FSH, (c + 1) * FSH)
        wsh = expert_W[:, :, sh]                       # [E, D, 128]
        ew8 = (wsh * SCALE_W).astype(f8)               # fp8, x64
        # corrT[f, d] = sum_e (W - W8/64)[e, d, f] / E
        s_err = wsh.sum(axis=0) - ew8.astype(np.float32).sum(axis=0) \
            / SCALE_W                                  # [D, 128]
        corrt = np.ascontiguousarray(s_err.T / E).astype(np.float32)

        # d >= DPE: [f, e] layout, grouped 4 d's per DMA
        wq = ew8[:, DPE:, :].transpose(1, 2, 0)        # [NDVE, 128, 1024]
        wq = np.ascontiguousarray(
            wq.reshape(NDVE // 4, 4, P, E).transpose(0, 2, 1, 3)
            .reshape(NDVE // 4, P, 4 * E))

        m = {
            "xt": xt,
            # roll shard columns to the front; softmax sum is invariant
            "gw": np.ascontiguousarray(
                np.roll(gate_W, -c * FSH, axis=1).astype(np.float16)),
            "ew8": np.ascontiguousarray(ew8[:, :DPE, :]),
            "wq8": wq,
            "corrt": corrt,
            # ebT[p, e] = expert_b[e, c*128 + p]
            "ebt": np.ascontiguousarray(
                expert_b[:, sh].T.astype(np.float16)),
        }
        if has_gb:
            m["gb"] = np.ascontiguousarray(
                np.roll(gate_b, -c * FSH, axis=1).astype(np.float16))
        in_maps.append(m)
    return in_maps, has_gb


def kernel(x, gate_W, gate_b, expert_W, expert_b, _trace=False):
    global LAST_RESULT
    from concourse.bass_utils import run_bass_kernel_spmd

    in_maps, has_gb = _prep_in_maps(x, gate_W, gate_b, expert_W, expert_b)

    key = ("nc", has_gb)
    if key not in _CACHE:
        _CACHE[key] = _build_bass(has_gb=has_gb)
    nc = _CACHE[key]

    res = run_bass_kernel_spmd(
        nc, in_maps, core_ids=list(range(NCORES)), trace=_trace,
    )
    LAST_RESULT = res

    out = np.empty([N, F], dtype=np.float32)
    for c in range(NCORES):
        out[:, c * FSH:(c + 1) * FSH] = \
            res.results[c]["outT"].astype(np.float32).T
    return out


# revision 17
# speedup vs baseline: 1.0860x; 1.0860x over previous
"""MoE layer kernel for 8x TRN2 NeuronCores (Bass/Tile).

Math (reference):
    w      = softmax(x @ gate_W + gate_b, axis=-1)[:E]          # [E, F]
    W_eff  = einsum('ef,edf->df', w, expert_W)                  # [D, F]
    b_eff  = einsum('ef,ef->f',  w, expert_b)                   # [F]
    out    = x @ W_eff + b_eff                                  # [N, F]

Sharding: F-parallel across 8 cores (each core owns 128 f-columns).

Key measured facts this implementation is built around:
  - fp16-stationary x fp8e4m3-moving PE matmul is exact; 512-col moving
    costs 213 ns at the full 2.4 GHz pstate when streamed gaplessly.
  - Each dma_start costs ~0.6-0.7 us of issuing-sequencer time, and a
    consumer can only start when its WHOLE transfer lands -> the
    expert_W stream uses per-e-tile (256 KB) DMAs on the Sync ring:
    fine enough to pipeline within a d-super-group, coarse enough that
    the issue rate (8 x 0.61 us per dsg) outruns PE (6.8 us per dsg).
  - The Scalar ring carries NO DMAs (the softmax exp chain paces the
    gate; DMA triggers on its sequencer would delay it by ~0.6 us
    each).  xT-rest rides the Vector ring; expert_b/corr/wq ride the
    otherwise idle GpSimd ring.
  - scalar_tensor_tensor has NO DVE perf modes -> fp8 in0 costs the
    same as fp16; one [128f, 1024e] STT per d (~1.13 us) computes a
    W_eff^T column outright.  NDVE=16 tail d's go this way, leaving
    DVE (~93 us) comfortably under PE so the diagonal-extract chase
    never stalls PE's PSUM-bank recycling.
  - fp8 systematic quantization error is removed by a host-precomputed
    correction: corrT[f, d] = sum_e(W - W8)[d, f] / E (w_e ~ 1/E makes
    sum_e w_e (W - W8)_e ~ mean); residual fluctuation ~0.8% rel.

Device algorithm per core:
  1. Gate GEMM (tokens 0..1023 == experts) on PE from host-transposed
     xT + gw; exp on ACT in two 512-col psum halves (rsA+rsB summed on
     DVE); normalize -> wnorm16 [e_p, et, f] fp16; wnT16 [f_p, e] via
     PE transposes through a 1-bank psum ring.  dsg0 of the diag phase
     is emitted et-major, interleaved into the gate, so PE fills the
     softmax-chain latency with diag work.
  2. W_eff^T, d < DPE: per d, psum[f, f'] = sum_e wnorm[e, f] *
     ew8[e, d, f'] (8 e-tile matmuls; moving tile covers 4 d -> psum
     [128, 512]); DVE extracts each d's diagonal via STT against
     identity, dequant 1/64 folded into the scalar operand.
  3. W_eff^T, d >= DPE: one DVE STT per d (fp8 [f, e] tiles x wnT16),
     accum straight into wefft[:, d]; ~1 interleaved per dsg.
  4. b_eff: one DVE STT (ebT fp16 [f, e] x wnT16) -> beff_col.
  5. wefft += corrT; transpose -> W_eff [d_p, f] fp16; out^T =
     W_eff^T @ xT on PE; bias fused into the psum->SBUF copies
     (alternating ACT/DVE); chunked DMA out (fp16).

All phases share ONE PSUM pool (lp 2x1 + wp 5x1 + tp 1 = 8 banks) —
pool open/close barriers between phases cost ~3-5 us each otherwise.

NOTE: this walrus build rejects any instruction carrying more than ONE
semaphore wait ("Too many sync wait commands"). _split_multi_waits()
post-processes the scheduled program, hoisting extra waits onto
standalone EventSemaphore instructions on the same engine queue.
"""

import numpy as np

N, D, E, F = 4096, 256, 1024, 1024
NCORES = 8
FSH = F // NCORES  # 128 f-columns per core
P = 128
SCALE_W = 64.0     # global fp8 scale for expert_W
DPE = 240          # d < DPE on the PE diag path (15 dsgs of 16)
NDVE = D - DPE     # 16 d's on the DVE STT path

_CACHE = {}
LAST_RESULT = None


def _split_multi_waits(nc):
    """Split multi-wait instructions into chains of single-wait ones."""
    import concourse.mybir as mybir

    n = 0
    for fn in nc.m.functions:
        for bb in fn.blocks:
            out = []
            changed = False
            for ins in bb.instructions:
                si = ins.sync_info
                if si is not None and si.on_wait and len(si.on_wait) > 1:
                    waits = list(si.on_wait)
                    for w in waits[:-1]:
                        es = mybir.InstEventSemaphore(
                            name=f"wsplit_{n}",
                            engine=ins.engine,
                            sync_info=mybir.SyncInfo(
                                on_wait=[w], on_update=[]),
                        )
                        out.append(es)
                        n += 1
                    ins.sync_info = mybir.SyncInfo(
                        on_wait=[waits[-1]], on_update=list(si.on_update))
                    changed = True
                out.append(ins)
            if changed:
                bb.instructions = out
    return n


def _build_bass(has_gb=False, dsg_size=16, prefetch_dsgs=5):
    import concourse.bass as bass
    import concourse.mybir as mybir
    from concourse.masks import make_identity
    from concourse.tile import TileContext

    f32 = mybir.dt.float32
    f16 = mybir.dt.float16
    f8 = mybir.dt.float8e4
    AF = mybir.ActivationFunctionType
    mult = mybir.AluOpType.mult
    add = mybir.AluOpType.add

    nc = bass.Bass(trn_type="TRN2", name="moe_fshard",
                   dynamic_dma_scratch_size=4096)

    xt_d = nc.dram_tensor("xt", [P, 2 * N], f16, kind="ExternalInput")
    gw_d = nc.dram_tensor("gw", [D, F], f16, kind="ExternalInput")
    if has_gb:
        gb_d = nc.dram_tensor("gb", [1, F], f16, kind="ExternalInput")
    ew_d = nc.dram_tensor("ew8", [E, DPE, FSH], f8, kind="ExternalInput")
    wq_d = nc.dram_tensor("wq8", [NDVE // 4, P, 4 * E], f8,
                          kind="ExternalInput")
    corr_d = nc.dram_tensor("corrt", [P, D], f32, kind="ExternalInput")
    eb_d = nc.dram_tensor("ebt", [P, E], f16, kind="ExternalInput")
    out_d = nc.dram_tensor("outT", [FSH, N], f16, kind="ExternalOutput")

    EBLK = E // P            # 8 e-tiles (gate token blocks)
    DH = D // P              # 2 halves of d
    NDSG = DPE // dsg_size   # 15 d super-groups on PE
    NBANK = dsg_size * P // 512   # 4 psum banks per super-group (4 d each)
    GN = EBLK * P            # 1024 gate tokens
    NWQ = NDVE // 4          # 4 wq tile groups

    with TileContext(nc) as tc:
        with tc.tile_pool(name="persist", bufs=1) as persist, \
             tc.tile_pool(name="wep", bufs=1 + prefetch_dsgs) as wep, \
             tc.tile_pool(name="wqp", bufs=NWQ) as wqp, \
             tc.tile_pool(name="ps8", bufs=1, space="PSUM") as ps8:

            # smalls: [:,0:128] identity f32; [:,128:129] beff_col
            smalls = persist.tile([P, 160], f32)
            ident = smalls[:, 0:128]
            beff_col = smalls[:, 128:129]
            sm16 = persist.tile([P, P], f16)
            ident16 = sm16[:, 0:P]

            xT = persist.tile([P, DH, N], f16)          # 16KB/part
            gw_sb = persist.tile([P, DH, F], f16)       # 4KB/part
            wnorm16 = persist.tile([P, EBLK, FSH], f16)  # 2KB/part
            wnT16 = persist.tile([P, E], f16)           # 2KB/part
            wefft = persist.tile([P, D], f32)           # 1KB/part
            weffc = persist.tile([P, D], f32)           # 1KB/part
            corrT = persist.tile([P, D], f32)           # 1KB/part
            weff = persist.tile([P, DH, FSH], f16)      # 0.5KB/part
            ebT_sb = persist.tile([P, E], f16)          # 2KB/part
            scr = persist.tile([P, 4], f32)
            rsA = scr[:, 0:1]
            rsB = scr[:, 1:2]
            rsum = scr[:, 2:3]
            rcp = scr[:, 3:4]
            expsc = persist.tile([P, F], f32)           # 4KB/part
            junk = persist.tile([P, P], f32)
            junk16 = persist.tile([P, E], f16)
            outT_sb = persist.tile([P, N], f16)         # 8KB/part
            if has_gb:
                gb_sb = persist.tile([1, F], f16)
                ones16 = persist.tile([1, P], f16)

            # ---- Sync ring: gate-critical inputs, then the ew8 stream
            nc.sync.dma_start(out=xT[:, 0, 0:GN], in_=xt_d[:, 0:GN])
            nc.sync.dma_start(
                out=gw_sb[:], in_=gw_d.rearrange("(h p) f -> p h f", p=P))
            nc.sync.dma_start(out=xT[:, 1, 0:GN],
                              in_=xt_d[:, N:N + GN])
            if has_gb:
                nc.sync.dma_start(out=gb_sb[:], in_=gb_d[:, :])

            wet_tiles = {}
            ew_v = ew_d.rearrange("(t p) d f -> p t d f", p=P)

            def issue_dsg(dsg, nsplit=1):
                w = wep.tile([P, EBLK, dsg_size, FSH], f8, tag="we",
                             name=f"wet_{dsg}")
                src = ew_v[:, :, dsg * dsg_size:(dsg + 1) * dsg_size, :]
                step = EBLK // nsplit
                for s in range(nsplit):
                    nc.sync.dma_start(out=w[:, s * step:(s + 1) * step],
                                      in_=src[:, s * step:(s + 1) * step])
                wet_tiles[dsg] = w

            issue_dsg(0, nsplit=4)
            for dsg in range(1, prefetch_dsgs):
                issue_dsg(dsg)

            # ---- GpSimd ring: corrT + the wq stream (expert_b after
            # the wq tiles: its consumer runs at the end of the diag)
            nc.gpsimd.dma_start(out=corrT[:], in_=corr_d[:, :])
            wq_tiles = {}

            def issue_wq(g):
                w = wqp.tile([P, 4, E], f8, tag="wq", name=f"wq_{g}")
                nc.gpsimd.dma_start(out=w[:], in_=wq_d[g, :, :])
                wq_tiles[g] = w

            issue_wq(0)
            issue_wq(1)
            nc.gpsimd.dma_start(out=ebT_sb[:], in_=eb_d[:, :])

            # constants (gpsimd iota), after the DMA issues
            make_identity(nc, ident)
            nc.scalar.copy(ident16[:], ident)
            if has_gb:
                nc.vector.memset(ones16[:], 1.0)

            def dve_stt(j):
                """DVE einsum for d = DPE + j."""
                nc.vector.scalar_tensor_tensor(
                    out=junk16[:],
                    in0=wq_tiles[j // 4][:, j % 4, :],
                    scalar=1.0 / SCALE_W, in1=wnT16[:],
                    op0=mult, op1=mult,
                    accum_out=wefft[:, DPE + j:DPE + j + 1])

            def extract(bank, di, d):
                # out shares junk16 with the dve_stt/beff ops: the WAW
                # chain pins the Tile scheduler to emission order on DVE
                # (otherwise it hoists the psum-independent STTs ahead
                # of the extracts and stalls PE's bank recycling).
                nc.vector.scalar_tensor_tensor(
                    out=junk16[:, 0:P],
                    in0=bank[:, di * P:(di + 1) * P],
                    scalar=1.0 / SCALE_W, in1=ident,
                    op0=mult, op1=mult,
                    accum_out=wefft[:, d:d + 1])

            # ====== Phase 1+2 interleaved: gate + diag dsg0 =========
            ETLAG = 4
            banks0 = [ps8.tile([P, 512], f32, tag="wp", bufs=5,
                               name=f"wp_0_{b}") for b in range(NBANK)]

            def diag_et(banks, wet, et):
                for b in range(NBANK):
                    nc.tensor.matmul(
                        banks[b][:],
                        wnorm16[:, et, :],
                        wet[:, et, b * 4:(b + 1) * 4, :],
                        start=(et == 0),
                        stop=(et == EBLK - 1))

            def wn_transpose(a):
                tp = ps8.tile([P, P], f16, tag="tp", bufs=1,
                              name=f"tp_{a}")
                nc.tensor.transpose(tp[:], wnorm16[:, a, :], ident16)
                nc.scalar.copy(wnT16[:, a * P:(a + 1) * P], tp[:])

            for a in range(EBLK):
                asl = slice(a * P, (a + 1) * P)
                lpA = ps8.tile([P, 512], f32, tag="lp", bufs=2,
                               name=f"lpA_{a}")
                lpB = ps8.tile([P, 512], f32, tag="lp", bufs=2,
                               name=f"lpB_{a}")
                for lp, cc in ((lpA, 0), (lpB, 512)):
                    nc.tensor.matmul(lp[:],
                                     xT[:, 0, asl],
                                     gw_sb[:, 0, cc:cc + 512],
                                     start=True, stop=False)
                    nc.tensor.matmul(lp[:],
                                     xT[:, 1, asl],
                                     gw_sb[:, 1, cc:cc + 512],
                                     start=False, stop=not has_gb)
                    if has_gb:
                        nc.tensor.matmul(lp[:], ones16[:],
                                         gb_sb[0:1, cc:cc + 512],
                                         start=False, stop=True)
                nc.scalar.activation(expsc[:, 0:512], lpA[:], AF.Exp,
                                     accum_out=rsA)
                nc.scalar.activation(expsc[:, 512:1024], lpB[:], AF.Exp,
                                     accum_out=rsB)
                nc.vector.scalar_tensor_tensor(
                    out=rsum, in0=rsA, scalar=1.0, in1=rsB,
                    op0=mult, op1=add)
                nc.vector.reciprocal(rcp, rsum)
                nc.vector.tensor_scalar_mul(
                    wnorm16[:, a, :], expsc[:, 0:FSH], rcp)
                if a >= ETLAG:
                    diag_et(banks0, wet_tiles[0], a - ETLAG)
                if a >= 2:
                    wn_transpose(a - 2)
            for et in range(EBLK - ETLAG, EBLK):
                diag_et(banks0, wet_tiles[0], et)
            wn_transpose(EBLK - 2)
            wn_transpose(EBLK - 1)
            wet_tiles.pop(0)

            ndve_done = 0

            def bank_finish(banks, dsg):
                nonlocal ndve_done
                for b in range(NBANK):
                    for di in range(4):
                        extract(banks[b], di,
                                dsg * dsg_size + b * 4 + di)
                if dsg < 3:
                    tgt = 0
                else:
                    tgt = min(NDVE,
                              ((dsg - 2) * NDVE) // (NDSG - 3))
                while ndve_done < tgt:
                    dve_stt(ndve_done)
                    g = ndve_done // 4
                    if g + 2 < NWQ and ndve_done % 4 == 0:
                        issue_wq(g + 2)
                    ndve_done += 1

            bank_finish(banks0, 0)

            # ====== Phase 2 cont.: dsg 1..14, bank-major ============
            for dsg in range(1, NDSG):
                if dsg + prefetch_dsgs <= NDSG:
                    issue_dsg(dsg + prefetch_dsgs - 1)
                if dsg == 6:
                    nc.sync.dma_start(out=xT[:, 0, GN:N],
                                      in_=xt_d[:, GN:N])
                if dsg == 10:
                    nc.sync.dma_start(out=xT[:, 1, GN:N],
                                      in_=xt_d[:, N + GN:2 * N])
                banks = [ps8.tile([P, 512], f32, tag="wp", bufs=5,
                                  name=f"wp_{dsg}_{b}")
                         for b in range(NBANK)]
                wet = wet_tiles.pop(dsg)
                for b in range(NBANK):
                    for et in range(EBLK):
                        nc.tensor.matmul(
                            banks[b][:],
                            wnorm16[:, et, :],
                            wet[:, et, b * 4:(b + 1) * 4, :],
                            start=(et == 0),
                            stop=(et == EBLK - 1))
                bank_finish(banks, dsg)
                if dsg == 8:
                    # d < 128 fully extracted: do half the correction +
                    # transpose now, shortening the tail
                    nc.vector.scalar_tensor_tensor(
                        out=weffc[:, 0:P], in0=wefft[:, 0:P],
                        scalar=1.0, in1=corrT[:, 0:P],
                        op0=mult, op1=add)
                    pt3e = ps8.tile([P, 512], f32, tag="wp", bufs=5,
                                    name="pt3_early")
                    nc.tensor.transpose(
                        pt3e[:, 0:P], weffc[:, 0:P], ident)
                    nc.scalar.copy(weff[:, 0, :], pt3e[:, 0:P])
            while ndve_done < NDVE:
                dve_stt(ndve_done)
                ndve_done += 1

            # b_eff: one DVE STT against wnT16 (only needed by the
            # GEMM-copy bias, so it runs after the extract chase)
            nc.vector.scalar_tensor_tensor(
                out=junk16[:], in0=ebT_sb[:], scalar=1.0, in1=wnT16[:],
                op0=mult, op1=mult, accum_out=beff_col)

            # fp8 mean-error correction (half 1; half 0 done mid-diag)
            nc.vector.scalar_tensor_tensor(
                out=weffc[:, P:D], in0=wefft[:, P:D], scalar=1.0,
                in1=corrT[:, P:D], op0=mult, op1=add)

            # ====== Phase 3: W_eff transpose + final GEMM ==========
            pt3 = ps8.tile([P, 512], f32, tag="wp", bufs=5,
                           name="pt3_1")
            nc.tensor.transpose(pt3[:, 0:P], weffc[:, P:D], ident)
            nc.scalar.copy(weff[:, 1, :], pt3[:, 0:P])
            for ch in range(N // 512):
                sl = slice(ch * 512, (ch + 1) * 512)
                ps = ps8.tile([P, 512], f32, tag="wp", bufs=5,
                              name=f"fp_{ch}")
                nc.tensor.matmul(ps[:], weff[:, 0, :],
                                 xT[:, 0, sl],
                                 start=True, stop=False)
                nc.tensor.matmul(ps[:], weff[:, 1, :],
                                 xT[:, 1, sl],
                                 start=False, stop=True)
                if ch % 2 == 0:
                    nc.scalar.activation(outT_sb[:, sl], ps[:],
                                         AF.Identity, bias=beff_col,
                                         scale=1.0)
                else:
                    nc.vector.tensor_scalar_add(
                        out=outT_sb[:, sl], in0=ps[:],
                        scalar1=beff_col)
                nc.sync.dma_start(out=out_d[:, sl],
                                  in_=outT_sb[:, sl])

    _split_multi_waits(nc)
    return nc


def _prep_in_maps(x, gate_W, gate_b, expert_W, expert_b):
    import ml_dtypes
    f8 = ml_dtypes.float8_e4m3

    x16 = np.asarray(x).astype(np.float16)
    # host transpose: xT[p, h, n] = x[n, h*128 + p]
    xt = np.ascontiguousarray(
        x16.T.reshape(2, P, N).transpose(1, 0, 2).reshape(P, 2 * N))
    gate_W = np.asarray(gate_W, dtype=np.float32)
    gate_b = np.asarray(gate_b, dtype=np.float32).reshape(1, F)
    has_gb = bool(np.any(gate_b))
    expert_W = np.asarray(expert_W, dtype=np.float32)
    expert_b = np.asarray(expert_b, dtype=np.float32)

    in_maps = []
    for c in range(NCORES):
        sh = slice(c * FSH, (c + 1) * FSH)
        wsh = expert_W[:, :, sh]                       # [E, D, 128]
        ew8 = (wsh * SCALE_W).astype(f8)               # fp8, x64
        # corrT[f, d] = sum_e (W - W8/64)[e, d, f] / E
        s_err = wsh.sum(axis=0) - ew8.astype(np.float32).sum(axis=0) \
            / SCALE_W                                  # [D, 128]
        corrt = np.ascontiguousarray(s_err.T / E).astype(np.float32)

        # d >= DPE: [f, e] layout, grouped 4 d's per DMA
        wq = ew8[:, DPE:, :].transpose(1, 2, 0)        # [NDVE, 128, 1024]
        wq = np.ascontiguousarray(
            wq.reshape(NDVE // 4, 4, P, E).transpose(0, 2, 1, 3)
            .reshape(NDVE // 4, P, 4 * E))

        m = {
            "xt": xt,
            # roll shard columns to the front; softmax sum is invariant
            "gw": np.ascontiguousarray(
                np.roll(gate_W, -c * FSH, axis=1).astype(np.float16)),
            "ew8": np.ascontiguousarray(ew8[:, :DPE, :]),
            "wq8": wq,
            "corrt": corrt,
            # ebT[p, e] = expert_b[e, c*128 + p]
            "ebt": np.ascontiguousarray(
                expert_b[:, sh].T.astype(np.float16)),
        }
        if has_gb:
            m["gb"] = np.ascontiguousarray(
                np.roll(gate_b, -c * FSH, axis=1).astype(np.float16))
        in_maps.append(m)
    return in_maps, has_gb


def kernel(x, gate_W, gate_b, expert_W, expert_b, _trace=False):
    global LAST_RESULT
    from concourse.bass_utils import run_bass_kernel_spmd

    in_maps, has_gb = _prep_in_maps(x, gate_W, gate_b, expert_W, expert_b)

    key = ("nc", has_gb)
    if key not in _CACHE:
        _CACHE[key] = _build_bass(has_gb=has_gb)
    nc = _CACHE[key]

    res = run_bass_kernel_spmd(
        nc, in_maps, core_ids=list(range(NCORES)), trace=_trace,
    )
    LAST_RESULT = res

    out = np.empty([N, F], dtype=np.float32)
    for c in range(NCORES):
        out[:, c * FSH:(c + 1) * FSH] = \
            res.results[c]["outT"].astype(np.float32).T
    return out
